# revision 36
# baseline (speedup 1.0000x reference)
"""Trainium2 Bass kernel for nn_MultiHeadedAttention (B=2, S=2048, D=1024, H=16).

Sharding: batch (2) x head-groups (4) -> 8 cores. Core c handles batch c//4,
heads [4*(c%4), 4*(c%4)+4). Heads are processed in PAIRS living on partition
halves: even head of a pair on partitions 0-63, odd head on 64-127, so the
scores matmuls (contraction dk=64) run as two concurrent row-tiled matmuls
(tile_position (0,0) / (64,0)) -> ~2x TensorE throughput for scores.

Phases (single unified PSUM pool so the tile scheduler can overlap them):
  1. K^T/Q^T projections into pair layout; V in [seq, feat] layout with a
     ones column per head (PV then yields softmax denominators for free).
     Projection bias-add + PSUM evacuation on ScalarE (idle during phase 1).
  2. Per (pair, J-block of 512, k-tile): row-tiled score pair -> exp ->
     PV. exp is split between ScalarE (true Exp activation) and VectorE
     (Schraudolph bit-trick: P_bits = round(s*a+b) as int16, bitcast bf16)
     so neither engine is the bottleneck. Softmax normalization uses
     reciprocal + DRAM-bounce partition broadcast + one vector multiply.
  3. y^T partial = Wo_s^T x_attn^T, emitted per J-block (bf16); host sums
     the 4 partials per batch and adds the output bias.

Matmul operands are bf16 (fp32 PSUM accumulation).
The tiny t-bias MLP ([B,1,1,1] -> [B,64]) is folded into the K projection
bias on the host during input sharding.
"""

import numpy as np

B, S, D, H, DK = 2, 2048, 1024, 16, 64
HPC = 4            # heads per core
NPAIR = HPC // 2   # head pairs per core
DPC = HPC * DK     # 256 features per core
NCORES = 8

JW = 512           # phase-2 J-block width
NJ = S // JW
NST = S // 128     # 16 k-tiles of 128
NE = D // 128      # 8 feature chunks (contraction for projections)
BW = 1024          # phase-1 projection block width
NB = S // BW

# Schraudolph fast-exp in bf16 bit space: bits = round(s*A + B) as int16,
# reinterpret as bf16 ~= exp(0.125*s). A = 0.125*log2(e)*2^7;
# B = 127*2^7 - sigma, sigma tuned to minimize max relative error (~3%).
SCH_A = 0.125 * 1.4426950408889634 * 128.0
SCH_B = 127.0 * 128.0 - 7.40
# k-tiles whose exp runs on VectorE (rest on ScalarE)
DVE_KTS = frozenset((2, 5, 8, 10, 12, 14))

TRACE = False          # test harness sets True to capture an NTFF profile
LAST_EXEC_NS = None    # filled when TRACE
LAST_RESULTS = None

_BUILT = None


def _install_ntff_shim():
    """antenv.axon_hooks is absent in this image; recreate it so trace=True
    can ship NTFF profiles back through the axon tunnel."""
    import sys, types
    try:
        from antenv import axon_hooks  # noqa: F401
        return
    except ImportError:
        pass
    import antenv
    mod = types.ModuleType("antenv.axon_hooks")
    _hook = [None]
    mod.set_axon_ntff_profile_hook = lambda h: _hook.__setitem__(0, h)
    mod.get_axon_ntff_profile_hook = lambda: _hook[0]
    sys.modules["antenv.axon_hooks"] = mod
    antenv.axon_hooks = mod
    try:
        from trn_agent_boot.trn_boot import _ntff_profile_via_ctypes
        mod.set_axon_ntff_profile_hook(
            _ntff_profile_via_ctypes("/opt/axon/libaxon_pjrt.so"))
    except Exception:
        pass


def _build():
    """Build the per-core Bass graph (identical on all 8 cores)."""
    import concourse.tile as tile
    from concourse import mybir, bacc

    f32 = mybir.dt.float32
    bf16 = mybir.dt.bfloat16
    i16 = mybir.dt.int16
    Exp = mybir.ActivationFunctionType.Exp
    Ident = mybir.ActivationFunctionType.Identity
    Copy = mybir.ActivationFunctionType.Copy

    nc = bacc.Bacc()

    xq_t = nc.dram_tensor("xq_t", [D, S], bf16, kind="ExternalInput")
    xk_t = nc.dram_tensor("xk_t", [D, S], bf16, kind="ExternalInput")
    xv_t = nc.dram_tensor("xv_t", [D, S], bf16, kind="ExternalInput")
    wq_t = nc.dram_tensor("wq_t", [D, DPC], bf16, kind="ExternalInput")
    wk_t = nc.dram_tensor("wk_t", [D, DPC], bf16, kind="ExternalInput")
    wv_t = nc.dram_tensor("wv_t", [D, DPC], bf16, kind="ExternalInput")
    wo_t = nc.dram_tensor("wo_t", [DPC, D], bf16, kind="ExternalInput")
    bq2 = nc.dram_tensor("bq2", [NPAIR, 128], f32, kind="ExternalInput")
    bk2 = nc.dram_tensor("bk2", [NPAIR, 128], f32, kind="ExternalInput")
    bv1 = nc.dram_tensor("bv1", [1, DPC], f32, kind="ExternalInput")
    y_t = nc.dram_tensor("y_t", [D, S], bf16, kind="ExternalOutput")

    with tile.TileContext(nc) as tc:
        with tc.tile_pool(name="consts", bufs=1) as consts, \
             tc.tile_pool(name="persist", bufs=1) as persist, \
             tc.tile_pool(name="xin", bufs=2) as xin, \
             tc.tile_pool(name="pp", bufs=4) as pp, \
             tc.tile_pool(name="oasb", bufs=2) as oa_pool, \
             tc.tile_pool(name="rsb", bufs=4) as r_pool, \
             tc.tile_pool(name="rbsb", bufs=4) as rb_pool, \
             tc.tile_pool(name="xasb", bufs=2) as xa_pool, \
             tc.tile_pool(name="ysb", bufs=4) as y_pool, \
             tc.tile_pool(name="drs", bufs=2, space="DRAM") as dr_pool, \
             tc.tile_pool(name="ps", bufs=1, space="PSUM") as ps_pool:

            # ---- constants ----
            # (weight/bias loads are interleaved with the x loads below so
            # the first projection chain starts as early as possible)
            wq_sb = consts.tile([128, NE, DPC], bf16, tag="wq")
            wk_sb = consts.tile([128, NE, DPC], bf16, tag="wk")
            wv_sb = consts.tile([128, NE, DPC], bf16, tag="wv")
            wo_sb = consts.tile([128, 2, D], bf16, tag="wo")
            bq_sb = consts.tile([128, NPAIR], f32, tag="bq")
            bk_sb = consts.tile([128, NPAIR], f32, tag="bk")
            bv_bc = consts.tile([128, HPC, DK], f32, tag="bvb")

            # ---- persistent activations ----
            # pair layout: partitions 0-63 = head 2m, 64-127 = head 2m+1
            qt_sb = persist.tile([128, NPAIR, S], bf16, tag="qt")
            kt_sb = persist.tile([128, NPAIR, S], bf16, tag="kt")
            # V per k-tile / head, plus a ones column (softmax denominators)
            v_sb = persist.tile([128, NST, HPC, DK + 1], bf16, tag="v")
            ones1 = consts.tile([128, 1], f32, tag="ones1")
            nc.vector.memset(ones1[:, :], 1.0)
            nc.vector.tensor_copy(
                v_sb[:, :, :, DK:DK + 1].rearrange("p a b c -> p (a b c)"),
                ones1[:, 0:1].broadcast_to([128, NST * HPC]))

            # ================= phase 1: projections =================
            def load_x(xdram, qb, tag, nchunk=2):
                # nchunk trades DMA-queue issue time (~0.65us/descriptor)
                # against how soon the first e-chunk is available
                xt = xin.tile([128, NE, BW], bf16, tag=tag)
                src = xdram.rearrange("(e p) s -> p e s", p=128)
                qs = slice(qb * BW, (qb + 1) * BW)
                estep = NE // nchunk
                for c in range(nchunk):
                    es = slice(c * estep, (c + 1) * estep)
                    nc.sync.dma_start(xt[:, es, :], src[:, es, qs])
                return xt

            def qk_chain(xt, qb, m, wsb, bsb, dst):
                # 128-feature chunk m (= head pair m) of a Q/K projection
                qs = slice(qb * BW, (qb + 1) * BW)
                ms = slice(m * 128, (m + 1) * 128)
                ps = ps_pool.tile([128, BW], f32, tag="big", bufs=2)
                for e in range(NE):
                    for hf in range(2):
                        hs = slice(hf * 512, hf * 512 + 512)
                        nc.tensor.matmul(ps[:, hs], wsb[:, e, ms],
                                         xt[:, e, hs],
                                         start=(e == 0), stop=(e == NE - 1))
                # bias-add + evacuation on ScalarE (idle during phase 1)
                nc.scalar.activation(dst[:, m, qs], ps[:, :], Ident,
                                     bias=bsb[:, m:m + 1])

            def v_chain(xt, qb, st):
                # k-tile st (within block qb) of the V projection
                stg = qb * (BW // 128) + st
                ps = ps_pool.tile([128, DPC], f32, tag="big", bufs=2)
                for e in range(NE):
                    nc.tensor.matmul(
                        ps[:, :],
                        xt[:, e, st * 128:(st + 1) * 128],
                        wv_sb[:, e, :],
                        start=(e == 0), stop=(e == NE - 1))
                nc.vector.tensor_tensor(
                    out=v_sb[:, stg, :, 0:DK],
                    in0=ps.rearrange("p (h d) -> p h d", h=HPC),
                    in1=bv_bc[:, :, :],
                    op=mybir.AluOpType.add)

            # K for all of S first (phase 2 needs full K^T), then per-block
            # V and Q.  DMA issue order is tuned: the Sync queue issues one
            # descriptor per ~0.6us, so the first chain's operands go first.
            wk_src = wk_t.rearrange("(e p) n -> p e n", p=128)
            nc.sync.dma_start(wk_sb[:, 0:1, :], wk_src[:, 0:1, :])
            nc.sync.dma_start(bk_sb[:, :], bk2.rearrange("m p -> p m"))
            nc.sync.dma_start(wk_sb[:, 1:NE, :], wk_src[:, 1:NE, :])
            xk0 = load_x(xk_t, 0, "xk", nchunk=8)
            xk1 = load_x(xk_t, 1, "xk", nchunk=4)
            nc.sync.dma_start(wq_sb[:, :, :],
                              wq_t.rearrange("(e p) n -> p e n", p=128))
            nc.sync.dma_start(bq_sb[:, :], bq2.rearrange("m p -> p m"))
            xq0 = load_x(xq_t, 0, "xq")
            nc.sync.dma_start(wv_sb[:, :, :],
                              wv_t.rearrange("(e p) n -> p e n", p=128))
            nc.sync.dma_start(
                bv_bc.rearrange("p h d -> p (h d)"),
                bv1[0:1, :].broadcast_to([128, DPC]))
            xv0 = load_x(xv_t, 0, "xv")
            nc.sync.dma_start(wo_sb[:, :, :],
                              wo_t.rearrange("(f p) n -> p f n", p=128))
            xv1 = load_x(xv_t, 1, "xv")
            xq1 = load_x(xq_t, 1, "xq")
            for m in range(NPAIR):
                qk_chain(xk0, 0, m, wk_sb, bk_sb, kt_sb)
            for m in range(NPAIR):
                qk_chain(xk1, 1, m, wk_sb, bk_sb, kt_sb)
            for m in range(NPAIR):
                qk_chain(xq0, 0, m, wq_sb, bq_sb, qt_sb)
            for st in range(BW // 128):
                v_chain(xv0, 0, st)
            # V block 1 and Q block 1 are interleaved into the first
            # J-block's attention stream (phase 2) via hooks

            # ================= phase 2: attention + out proj =================
            LAG = 4   # PV trails scores/exp by this many k-tiles

            def attn_pair(p, J, xa, last=False, hooks=None):
                Js = slice(J * JW, (J + 1) * JW)
                o_ps = ps_pool.tile([DK + 1, 2, JW], f32, tag="ov", bufs=2)
                pbufs = [None] * NST

                def scores_exp(kt):
                    ks = slice(kt * 128, (kt + 1) * 128)
                    sc = ps_pool.tile([128, 2, JW], f32, tag="big", bufs=2)
                    nc.tensor.matmul(sc[:, 0, :], kt_sb[0:64, p, ks],
                                     qt_sb[0:64, p, Js],
                                     start=True, stop=True,
                                     tile_position=(0, 0))
                    nc.tensor.matmul(sc[:, 1, :], kt_sb[64:128, p, ks],
                                     qt_sb[64:128, p, Js],
                                     start=True, stop=True,
                                     tile_position=(64, 0))
                    if kt in DVE_KTS:
                        pi = pp.tile([128, 2, JW], i16, tag="pi", bufs=8)
                        nc.vector.tensor_scalar(
                            pi[:, :, :], sc[:, :, :], SCH_A, SCH_B,
                            mybir.AluOpType.mult, mybir.AluOpType.add)
                        pbufs[kt] = pi.bitcast(bf16)
                    else:
                        p_bf = pp.tile([128, 2, JW], bf16, tag="pb", bufs=8)
                        nc.scalar.activation(p_bf[:, :, :], sc[:, :, :],
                                             Exp, scale=0.125)
                        pbufs[kt] = p_bf

                def pv(kt):
                    for h2 in range(2):
                        nc.tensor.matmul(o_ps[:, h2, :],
                                         v_sb[:, kt, 2 * p + h2, :],
                                         pbufs[kt][:, h2, :],
                                         start=(kt == 0), stop=(kt == NST - 1))

                for kt in range(NST):
                    scores_exp(kt)
                    if hooks and kt in hooks:
                        hooks[kt]()
                    if kt >= LAG:
                        pv(kt - LAG)
                for kt in range(NST - LAG, NST):
                    pv(kt)
                # evacuate the pair accumulator, then normalize
                oa = oa_pool.tile([DK + 1, 2, JW], f32, tag="oa")
                if last:
                    nc.scalar.activation(oa[:, :, :], o_ps[:, :, :], Copy)
                else:
                    nc.vector.tensor_copy(oa[:, :, :], o_ps[:, :, :])
                # reciprocal of the denominators on a [128, ...] reshape
                # (lane-parallel) via a DRAM bounce
                dq = nc.sync if last else nc.gpsimd
                for h2 in range(2):
                    d1 = dr_pool.tile([1, JW], f32, tag="d1")
                    dq.dma_start(d1[0:1, :], oa[DK:DK + 1, h2, :])
                    r2 = r_pool.tile([128, JW // 128], f32, tag="r2")
                    dq.dma_start(
                        r2[:, :], d1.rearrange("o (p f) -> (o p) f", p=128))
                    nc.vector.reciprocal(r2[:, :], r2[:, :])
                    d2 = dr_pool.tile([128, JW // 128], f32, tag="d2")
                    dq.dma_start(d2[:, :], r2[:, :])
                    rb = rb_pool.tile([64, JW], f32, tag="rb")
                    dq.dma_start(
                        rb[:, :],
                        d2.rearrange("p f -> (p f)").unsqueeze(0)
                          .broadcast_to([64, JW]))
                    # GpSimd handles the steady-state multiplies; the final
                    # pair's are latency-critical, so they go to VectorE
                    eng = nc.vector if last else nc.gpsimd
                    eng.tensor_tensor(
                        out=xa[64 * h2:64 * h2 + 64, p, :],
                        in0=oa[0:DK, h2, :], in1=rb[:, :],
                        op=mybir.AluOpType.mult)

            def emit_y(J, xa, last=False):
                Js = slice(J * JW, (J + 1) * JW)
                for o in range(8):
                    os_ = slice(o * 128, (o + 1) * 128)
                    ps = ps_pool.tile([128, JW], f32, tag="ov", bufs=2)
                    for f in range(2):
                        nc.tensor.matmul(ps[:, :], wo_sb[:, f, os_],
                                         xa[:, f, :],
                                         start=(f == 0), stop=(f == 1))
                    y_sb = y_pool.tile([128, JW], bf16, tag="y")
                    if o % 2 == 0:
                        nc.vector.tensor_copy(y_sb[:, :], ps[:, :])
                        nc.sync.dma_start(y_t[os_, Js], y_sb[:, :])
                    else:
                        nc.scalar.activation(y_sb[:, :], ps[:, :], Copy)
                        (nc.gpsimd if last else nc.sync).dma_start(
                            y_t[os_, Js], y_sb[:, :])

            hook_kts = (1, 3, 5, 6, 7, 9, 11, 13)
            hooks_p0 = {kt: (lambda s=st: v_chain(xv1, 1, s))
                        for st, kt in enumerate(hook_kts)}
            hooks_p1 = {2: lambda: qk_chain(xq1, 1, 0, wq_sb, bq_sb, qt_sb),
                        10: lambda: qk_chain(xq1, 1, 1, wq_sb, bq_sb, qt_sb)}
            xa_prev = None
            for J in range(NJ):
                xa = xa_pool.tile([128, NPAIR, JW], bf16, tag="xa")
                for p in range(NPAIR):
                    attn_pair(p, J, xa, last=(J == NJ - 1 and p == NPAIR - 1),
                              hooks=(hooks_p0 if (J, p) == (0, 0) else
                                     hooks_p1 if (J, p) == (0, 1) else None))
                # previous J-block's out-projection fires after this block's
                # pairs are queued, so its xa is long since finalized
                if xa_prev is not None:
                    emit_y(J - 1, xa_prev)
                xa_prev = xa
            emit_y(NJ - 1, xa_prev, last=True)

    nc.finalize()
    return nc


def _get_built():
    global _BUILT
    if _BUILT is None:
        _BUILT = _build()
    return _BUILT


def kernel(**inputs):
    global LAST_EXEC_NS, LAST_RESULTS
    import ml_dtypes
    from concourse import bass_utils

    bf16 = ml_dtypes.bfloat16
    inp = {k: np.ascontiguousarray(np.asarray(v), dtype=np.float32)
           for k, v in inputs.items()}

    # host: t-bias MLP, folded into the K-projection bias
    t = inp["t"].reshape(B)
    h1 = np.maximum(inp["tW1"][:, 0][None, :] * t[:, None] + inp["tb1"][None, :], 0.0)
    t_bias = h1 @ inp["tW2"].T + inp["tb2"][None, :]          # [B, DK]

    in_maps = []
    for c in range(NCORES):
        b, g = c // 4, c % 4
        sl = slice(g * DPC, (g + 1) * DPC)
        in_maps.append({
            "xq_t": np.ascontiguousarray(inp["query"][b].T.astype(bf16)),
            "xk_t": np.ascontiguousarray(inp["key"][b].T.astype(bf16)),
            "xv_t": np.ascontiguousarray(inp["value"][b].T.astype(bf16)),
            "wq_t": np.ascontiguousarray(inp["Wq"][sl, :].T.astype(bf16)),
            "wk_t": np.ascontiguousarray(inp["Wk"][sl, :].T.astype(bf16)),
            "wv_t": np.ascontiguousarray(inp["Wv"][sl, :].T.astype(bf16)),
            "wo_t": np.ascontiguousarray(inp["Wo"][:, sl].T.astype(bf16)),
            "bq2": inp["bq"][sl].reshape(NPAIR, 128).copy(),
            "bk2": (inp["bk"][sl] + np.tile(t_bias[b], HPC)
                    ).reshape(NPAIR, 128),
            "bv1": inp["bv"][sl].reshape(1, DPC).copy(),
        })

    nc = _get_built()
    if TRACE:
        _install_ntff_shim()
    try:
        res = bass_utils.run_bass_kernel_spmd(
            nc, in_maps, core_ids=list(range(NCORES)), trace=TRACE)
    except Exception:
        # transient device-unrecoverable states have been observed on a
        # first run; one retry on a fresh execute context clears them
        import time
        time.sleep(2.0)
        res = bass_utils.run_bass_kernel_spmd(
            nc, in_maps, core_ids=list(range(NCORES)), trace=False)
    LAST_EXEC_NS = res.exec_time_ns
    LAST_RESULTS = res

    out = np.zeros((B, S, D), np.float32)
    for c in range(NCORES):
        out[c // 4] += res.results[c]["y_t"].T.astype(np.float32)
    out += inp["bo"][None, None, :]
    return out
